# revision 2
# baseline (speedup 1.0000x reference)
"""W4A16 column-parallel linear kernel for Trainium2 (8 NeuronCores).

y = x @ dequant(qweight_packed, w_scales).T + bias
  x: [4, 2048, 4096] f32
  qweight_packed: [11008, 2048] int32 (two int4 nibbles per byte, low first)
  w_scales: [11008, 1] f32, bias: [11008] f32
  -> y: [4, 2048, 11008] f32

Sharding: column-parallel over out_features (1376 rows of W per core).

Hybrid-precision matmul: K is split into 16 "byte-tiles" of 256 contraction
elements (128 packed bytes = 128 lo + 128 hi nibbles).  The first KB2
byte-tiles run as bf16 matmuls (2 MMs of K=128 each); the last KF run as
fp8e4 DoubleRow matmuls (1 MM of K=256 each, 2x FLOP rate).  int4 weight
values are exact in both bf16 and fp8e4; the only extra error is e4m3
quantization of x on the fp8 fraction, which the KF/KB2 split keeps under
the 2e-2 gate with margin.

Weights stay resident in SBUF (dequantized on device from packed nibbles);
x streams in per 256-token super-tile; f32 PSUM accumulation; scale+bias
applied on PSUM eviction by DVE.
"""

import os
import sys

import numpy as np
import ml_dtypes

for _p in ("/opt/trn_rl_repo", "/root/.axon_site/_ro/trn_rl_repo"):
    if os.path.isdir(_p) and _p not in sys.path:
        sys.path.append(_p)

import concourse.bacc as bacc
import concourse.tile as tile
import concourse.mybir as mybir
from concourse.bass_utils import run_bass_kernel_spmd

dt = mybir.dt
Alu = mybir.AluOpType
BF16 = ml_dtypes.bfloat16
FP8 = ml_dtypes.float8_e4m3  # IEEE e4m3 (max 240) == TRN FP8_EXP4

# Problem shape (hardcoded per harness contract)
B, S, K_FULL, N_FULL = 4, 2048, 4096, 11008
N_CORES = 8
M_FULL = B * S            # 8192
N_SH = N_FULL // N_CORES  # 1376
P = 128
M_SUP = 256               # tokens per x super-tile
NKT = K_FULL // 256       # 16 byte-tiles (256 contraction elems each)
KF = 10                   # byte-tiles computed in fp8 DoubleRow (last KF)
KB2 = NKT - KF            # byte-tiles computed in bf16 (first KB2)

LO_MASK = 0x0F0F0F0F
XOR8 = 0x08080808


def build_nc(M, NSH, kf=KF, m_sup=M_SUP, nb_max=512):
    """One core's module: y[M, NSH] = x[M, K] @ W[NSH, K].T (K = 256*NKT)."""
    kb2 = NKT - kf
    n_ms = M // m_sup
    n_mi = m_sup // P
    nbs = []
    off = 0
    while off < NSH:
        w = min(nb_max, NSH - off)
        nbs.append((off, w))
        off += w

    nc = bacc.Bacc("TRN2", target_bir_lowering=False, debug=False)
    # fp8 x tiles: [ms, i, p, j, m] = x[ms*m_sup+m, 256*(kb2+i) + 2p + j]
    xf = nc.dram_tensor("xf", [n_ms, kf, P, 2, m_sup], dt.float8e4,
                        kind="ExternalInput")
    # bf16 x tiles: [ms, i, j, p, m] = x[ms*m_sup+m, 256*i + 2p + j]
    xb = nc.dram_tensor("xb", [n_ms, kb2, 2, P, m_sup], dt.bfloat16,
                        kind="ExternalInput")
    qt = nc.dram_tensor("qt", [NKT, P, NSH], dt.int8, kind="ExternalInput")
    scb = nc.dram_tensor("scb", [P, NSH], dt.float32, kind="ExternalInput")
    bib = nc.dram_tensor("bib", [P, NSH], dt.float32, kind="ExternalInput")
    y = nc.dram_tensor("y", [M, NSH], dt.float32, kind="ExternalOutput")

    with tile.TileContext(nc) as tc:
        with (
            tc.tile_pool(name="wpool", bufs=1) as wpool,
            tc.tile_pool(name="qpool", bufs=3) as qpool,
            tc.tile_pool(name="xpool", bufs=2) as xpool,
            tc.tile_pool(name="cpool", bufs=1) as cpool,
            tc.tile_pool(name="opool", bufs=4) as opool,
            tc.tile_pool(name="pspool", bufs=8, space="PSUM") as pspool,
        ):
            m8 = cpool.tile([P, 1], dt.float32, tag="m8")
            nc.vector.memset(m8[:], -8.0)

            # Dequant: byte-tile ct covers k = 256ct + 2p + j.
            # ct < kb2 -> two bf16 planes; ct >= kb2 -> one fp8 [P,2,NSH].
            wb = [None] * (2 * kb2)
            w8 = [None] * kf
            for ct in range(NKT):
                u = qpool.tile([P, NSH], dt.int8, tag="q", name=f"u{ct}")
                nc.gpsimd.dma_start(u[:], qt[ct])
                u32 = u[:].bitcast(dt.int32)
                # lo nibble as (lo ^ 8) in [0,15]
                tl = qpool.tile([P, NSH], dt.int8, tag="tl", name=f"tl{ct}")
                nc.vector.tensor_scalar(tl[:].bitcast(dt.int32), u32,
                                        LO_MASK, XOR8,
                                        op0=Alu.bitwise_and,
                                        op1=Alu.bitwise_xor)
                # hi nibble as (hi ^ 8) in [0,15]
                ta = qpool.tile([P, NSH], dt.int8, tag="ta", name=f"ta{ct}")
                nc.vector.tensor_scalar(ta[:].bitcast(dt.int32), u32,
                                        4, LO_MASK,
                                        op0=Alu.logical_shift_right,
                                        op1=Alu.bitwise_and)
                th = qpool.tile([P, NSH], dt.int8, tag="th", name=f"th{ct}")
                nc.vector.tensor_scalar(th[:].bitcast(dt.int32),
                                        ta[:].bitcast(dt.int32),
                                        XOR8, None, op0=Alu.bitwise_xor)
                if ct < kb2:
                    wlo = wpool.tile([P, NSH], dt.bfloat16, tag=f"wb{2*ct}")
                    nc.scalar.activation(wlo[:], tl[:],
                                         mybir.ActivationFunctionType.Identity,
                                         bias=m8[:], scale=1.0)
                    whi = wpool.tile([P, NSH], dt.bfloat16,
                                     tag=f"wb{2*ct + 1}")
                    nc.vector.tensor_scalar(whi[:], th[:], -8, None,
                                            op0=Alu.add)
                    wb[2 * ct] = wlo
                    wb[2 * ct + 1] = whi
                else:
                    i = ct - kb2
                    wf = wpool.tile([P, 2, NSH], dt.float8e4, tag=f"w8{i}")
                    nc.scalar.activation(wf[:, 0, :], tl[:],
                                         mybir.ActivationFunctionType.Identity,
                                         bias=m8[:], scale=1.0)
                    nc.vector.tensor_scalar(wf[:, 1, :], th[:], -8, None,
                                            op0=Alu.add)
                    w8[i] = wf

            sc = cpool.tile([P, NSH], dt.float32, tag="sc")
            nc.gpsimd.dma_start(sc[:], scb[:])
            bi = cpool.tile([P, NSH], dt.float32, tag="bi")
            nc.gpsimd.dma_start(bi[:], bib[:])

            groups = [(mi, nb0, nbw) for mi in range(n_mi)
                      for nb0, nbw in nbs]
            n_mm = 2 * kb2 + kf  # MM issue slots per group

            def evict(ps, mi, nb0, nbw, ms):
                osb = opool.tile([P, nbw], dt.float32, tag="o")
                nc.vector.tensor_tensor(osb[:], ps[:], sc[:, nb0:nb0 + nbw],
                                        op=Alu.mult)
                nc.vector.tensor_tensor(osb[:], osb[:], bi[:, nb0:nb0 + nbw],
                                        op=Alu.add)
                r0 = ms * m_sup + mi * P
                nc.sync.dma_start(y[r0:r0 + P, nb0:nb0 + nbw], osb[:])

            def issue_mm(ps, ct, mi, nb0, nbw, xbi, xfi, start, stop):
                """MMs for byte-tile ct into psum ps. Returns #MMs issued."""
                if ct < kb2:
                    for jj in range(2):
                        t = 2 * ct + jj
                        nc.tensor.matmul(
                            ps[:],
                            xbi[t][:, mi * P:mi * P + P],
                            wb[t][:, nb0:nb0 + nbw],
                            start=start and jj == 0,
                            stop=stop and jj == 1,
                        )
                else:
                    i = ct - kb2
                    nc.tensor.matmul(
                        ps[:],
                        xfi[i][:, :, mi * P:mi * P + P],
                        w8[i][:, :, nb0:nb0 + nbw],
                        start=start,
                        stop=stop,
                        perf_mode=mybir.MatmulPerfMode.DoubleRow,
                    )

            for ms in range(n_ms):
                # x tile DMAs in consumption order (bf16 tiles then fp8)
                xbi = [None] * (2 * kb2)
                xfi = [None] * kf
                for ct in range(NKT):
                    if ct < kb2:
                        for jj in range(2):
                            t = 2 * ct + jj
                            xt_ = xpool.tile([P, m_sup], dt.bfloat16,
                                             tag=f"xb{t}", name=f"xb{ms}_{t}")
                            nc.sync.dma_start(xt_[:], xb[ms, ct, jj])
                            xbi[t] = xt_
                    else:
                        i = ct - kb2
                        xt_ = xpool.tile([P, 2, m_sup], dt.float8e4,
                                         tag=f"xf{i}", name=f"xf{ms}_{i}")
                        nc.sync.dma_start(xt_[:], xf[ms, i])
                        xfi[i] = xt_

                if ms == 0 and len(groups) <= 8:
                    # k-major across psum groups so PE consumes each W tile
                    # as dequant produces it instead of stalling on the set.
                    pss = [pspool.tile([P, nbw], dt.float32, tag="ps",
                                       name=f"ps{g}")
                           for g, (mi, nb0, nbw) in enumerate(groups)]
                    ng = len(groups)
                    for s in range(NKT):
                        for gi in range(ng):
                            g = (gi + s) % ng
                            mi, nb0, nbw = groups[g]
                            issue_mm(pss[g], s, mi, nb0, nbw, xbi, xfi,
                                     start=(s == 0), stop=(s == NKT - 1))
                    for g, (mi, nb0, nbw) in enumerate(groups):
                        evict(pss[g], mi, nb0, nbw, ms)
                else:
                    for mi, nb0, nbw in groups:
                        ps = pspool.tile([P, nbw], dt.float32, tag="ps")
                        for ct in range(NKT):
                            issue_mm(ps, ct, mi, nb0, nbw, xbi, xfi,
                                     start=(ct == 0), stop=(ct == NKT - 1))
                        evict(ps, mi, nb0, nbw, ms)

    nc.compile()
    return nc


def prep_x(x2, kf=KF, m_sup=M_SUP):
    """[M, K] f32 -> (xf fp8 [n_ms,kf,P,2,m_sup], xb bf16 [n_ms,kb2,2,P,m_sup])."""
    M, K = x2.shape
    kb2 = NKT - kf
    n_ms = M // m_sup
    # k = 256*ct + 2*p + j
    xv = x2.reshape(n_ms, m_sup, NKT, P, 2)
    xf = np.ascontiguousarray(
        xv[:, :, kb2:, :, :].transpose(0, 2, 3, 4, 1)).astype(FP8)
    xb = np.ascontiguousarray(
        xv[:, :, :kb2, :, :].transpose(0, 2, 4, 3, 1)).astype(BF16)
    return xf, xb


def prep_q(q_u8_shard):
    """[NSH, KP] uint8 -> [NKT, P, NSH] int8 (transposed packed bytes)."""
    NSH, KP = q_u8_shard.shape
    return np.ascontiguousarray(q_u8_shard.T).view(np.int8).reshape(
        KP // P, P, NSH)


def prep_bcast(v):
    """[NSH] f32 -> [P, NSH] f32 broadcast tile."""
    return np.ascontiguousarray(
        np.broadcast_to(v.astype(np.float32)[None, :], (P, v.shape[0])))


def _ensure_ntff_hook():
    """Register the axon NTFF profiling hook if the image's antenv lacks
    axon_hooks (trn_boot degrades silently in that case)."""
    try:
        from antenv.axon_hooks import get_axon_ntff_profile_hook  # noqa: F401
        return
    except ImportError:
        pass
    import types
    import antenv
    mod = types.ModuleType("antenv.axon_hooks")
    _h = {"hook": None}
    mod.set_axon_ntff_profile_hook = lambda h: _h.__setitem__("hook", h)
    mod.get_axon_ntff_profile_hook = lambda: _h["hook"]
    sys.modules["antenv.axon_hooks"] = mod
    antenv.axon_hooks = mod
    try:
        from trn_agent_boot.trn_boot import _ntff_profile_via_ctypes
        hook = _ntff_profile_via_ctypes("/opt/axon/libaxon_pjrt.so")
        if hook is not None:
            mod.set_axon_ntff_profile_hook(hook)
    except Exception as e:  # profiling optional; run still works
        print("ntff hook setup failed:", e)


_NC_CACHE = {}


def _get_nc():
    key = (M_FULL, N_SH, KF, M_SUP)
    if key not in _NC_CACHE:
        _NC_CACHE[key] = build_nc(M_FULL, N_SH, kf=KF, m_sup=M_SUP)
    return _NC_CACHE[key]


LAST_RESULT = None


def kernel(x, qweight_packed, w_scales, bias, _profile=False):
    global LAST_RESULT
    x = np.asarray(x)
    qweight_packed = np.asarray(qweight_packed)
    w_scales = np.asarray(w_scales)
    bias = np.asarray(bias)

    # Always shim the profiling hook module: run_bass_kernel_spmd imports
    # it whenever tracing is requested (including via env BASS_TRACE).
    _ensure_ntff_hook()

    nc = _get_nc()

    x2 = np.ascontiguousarray(x.reshape(M_FULL, K_FULL).astype(np.float32))
    xf, xb = prep_x(x2)
    q_u8 = qweight_packed.astype(np.uint8)
    scales_flat = w_scales.reshape(N_FULL)
    bias_flat = bias.reshape(N_FULL)

    in_maps = []
    for c in range(N_CORES):
        r0, r1 = c * N_SH, (c + 1) * N_SH
        in_maps.append({
            "xf": xf,
            "xb": xb,
            "qt": prep_q(q_u8[r0:r1]),
            "scb": prep_bcast(scales_flat[r0:r1]),
            "bib": prep_bcast(bias_flat[r0:r1]),
        })

    res = run_bass_kernel_spmd(nc, in_maps, list(range(N_CORES)),
                               trace=_profile)
    LAST_RESULT = res
    y = np.concatenate([res.results[c]["y"] for c in range(N_CORES)], axis=1)
    return y.reshape(B, S, N_FULL)


# revision 3
# speedup vs baseline: 1.0919x; 1.0919x over previous
"""W4A16 column-parallel linear kernel for Trainium2 (8 NeuronCores).

y = x @ dequant(qweight_packed, w_scales).T + bias
  x: [4, 2048, 4096] f32
  qweight_packed: [11008, 2048] int32 (two int4 nibbles per byte, low first)
  w_scales: [11008, 1] f32, bias: [11008] f32
  -> y: [4, 2048, 11008] f32

Sharding: column-parallel over out_features (1376 rows of W per core).

Hybrid-precision matmul: K is split into 16 "byte-tiles" of 256 contraction
elements (128 packed bytes = 128 lo + 128 hi nibbles).  The first KB2
byte-tiles run as bf16 matmuls (2 MMs of K=128 each); the last KF run as
fp8e4 DoubleRow matmuls (1 MM of K=256 each, 2x FLOP rate).  int4 weight
values are exact in both bf16 and fp8e4; the only extra error is e4m3
quantization of x on the fp8 fraction, which the KF/KB2 split keeps under
the 2e-2 gate with margin.

Weights stay resident in SBUF (dequantized on device from packed nibbles);
x streams in per 256-token super-tile; f32 PSUM accumulation; scale+bias
applied on PSUM eviction by DVE.
"""

import os
import sys

import numpy as np
import ml_dtypes

for _p in ("/opt/trn_rl_repo", "/root/.axon_site/_ro/trn_rl_repo"):
    if os.path.isdir(_p) and _p not in sys.path:
        sys.path.append(_p)

import concourse.bacc as bacc
import concourse.tile as tile
import concourse.mybir as mybir
from concourse.bass_utils import run_bass_kernel_spmd

dt = mybir.dt
Alu = mybir.AluOpType
BF16 = ml_dtypes.bfloat16
FP8 = ml_dtypes.float8_e4m3  # IEEE e4m3 (max 240) == TRN FP8_EXP4

# Problem shape (hardcoded per harness contract)
B, S, K_FULL, N_FULL = 4, 2048, 4096, 11008
N_CORES = 8
M_FULL = B * S            # 8192
N_SH = N_FULL // N_CORES  # 1376
P = 128
M_SUP = 256               # tokens per x super-tile
NKT = K_FULL // 256       # 16 byte-tiles (256 contraction elems each)
KF = 12                   # byte-tiles computed in fp8 DoubleRow (last KF)
KB2 = NKT - KF            # byte-tiles computed in bf16 (first KB2)

LO_MASK = 0x0F0F0F0F
XOR8 = 0x08080808


def build_nc(M, NSH, kf=KF, m_sup=M_SUP, nb_max=512):
    """One core's module: y[M, NSH] = x[M, K] @ W[NSH, K].T (K = 256*NKT)."""
    kb2 = NKT - kf
    n_ms = M // m_sup
    n_mi = m_sup // P
    nbs = []
    off = 0
    while off < NSH:
        w = min(nb_max, NSH - off)
        nbs.append((off, w))
        off += w

    nc = bacc.Bacc("TRN2", target_bir_lowering=False, debug=False)
    # fp8 x tiles: [ms, i, p, j, m] = x[ms*m_sup+m, 256*(kb2+i) + 2p + j]
    xf = nc.dram_tensor("xf", [n_ms, kf, P, 2, m_sup], dt.float8e4,
                        kind="ExternalInput")
    # bf16 x tiles: [ms, i, j, p, m] = x[ms*m_sup+m, 256*i + 2p + j]
    xb = nc.dram_tensor("xb", [n_ms, kb2, 2, P, m_sup], dt.bfloat16,
                        kind="ExternalInput")
    qt = nc.dram_tensor("qt", [NKT, P, NSH], dt.int8, kind="ExternalInput")
    scb = nc.dram_tensor("scb", [P, NSH], dt.float32, kind="ExternalInput")
    bib = nc.dram_tensor("bib", [P, NSH], dt.float32, kind="ExternalInput")
    y = nc.dram_tensor("y", [M, NSH], dt.float32, kind="ExternalOutput")

    with tile.TileContext(nc) as tc:
        with (
            tc.tile_pool(name="wpool", bufs=1) as wpool,
            tc.tile_pool(name="qpool", bufs=3) as qpool,
            tc.tile_pool(name="xpool", bufs=2) as xpool,
            tc.tile_pool(name="cpool", bufs=1) as cpool,
            tc.tile_pool(name="opool", bufs=4) as opool,
            tc.tile_pool(name="pspool", bufs=8, space="PSUM") as pspool,
        ):
            m8 = cpool.tile([P, 1], dt.float32, tag="m8")
            nc.vector.memset(m8[:], -8.0)

            # Dequant: byte-tile ct covers k = 256ct + 2p + j.
            # ct < kb2 -> two bf16 planes; ct >= kb2 -> one fp8 [P,2,NSH].
            wb = [None] * (2 * kb2)
            w8 = [None] * kf
            for ct in range(NKT):
                u = qpool.tile([P, NSH], dt.int8, tag="q", name=f"u{ct}")
                nc.gpsimd.dma_start(u[:], qt[ct])
                u32 = u[:].bitcast(dt.int32)
                # lo nibble as (lo ^ 8) in [0,15]
                tl = qpool.tile([P, NSH], dt.int8, tag="tl", name=f"tl{ct}")
                nc.vector.tensor_scalar(tl[:].bitcast(dt.int32), u32,
                                        LO_MASK, XOR8,
                                        op0=Alu.bitwise_and,
                                        op1=Alu.bitwise_xor)
                # hi nibble as (hi ^ 8) in [0,15]
                ta = qpool.tile([P, NSH], dt.int8, tag="ta", name=f"ta{ct}")
                nc.vector.tensor_scalar(ta[:].bitcast(dt.int32), u32,
                                        4, LO_MASK,
                                        op0=Alu.logical_shift_right,
                                        op1=Alu.bitwise_and)
                th = qpool.tile([P, NSH], dt.int8, tag="th", name=f"th{ct}")
                nc.vector.tensor_scalar(th[:].bitcast(dt.int32),
                                        ta[:].bitcast(dt.int32),
                                        XOR8, None, op0=Alu.bitwise_xor)
                if ct < kb2:
                    wlo = wpool.tile([P, NSH], dt.bfloat16, tag=f"wb{2*ct}")
                    nc.scalar.activation(wlo[:], tl[:],
                                         mybir.ActivationFunctionType.Identity,
                                         bias=m8[:], scale=1.0)
                    whi = wpool.tile([P, NSH], dt.bfloat16,
                                     tag=f"wb{2*ct + 1}")
                    nc.vector.tensor_scalar(whi[:], th[:], -8, None,
                                            op0=Alu.add)
                    wb[2 * ct] = wlo
                    wb[2 * ct + 1] = whi
                else:
                    i = ct - kb2
                    wf = wpool.tile([P, 2, NSH], dt.float8e4, tag=f"w8{i}")
                    nc.scalar.activation(wf[:, 0, :], tl[:],
                                         mybir.ActivationFunctionType.Identity,
                                         bias=m8[:], scale=1.0)
                    nc.vector.tensor_scalar(wf[:, 1, :], th[:], -8, None,
                                            op0=Alu.add)
                    w8[i] = wf

            sc = cpool.tile([P, NSH], dt.float32, tag="sc")
            nc.gpsimd.dma_start(sc[:], scb[:])
            bi = cpool.tile([P, NSH], dt.float32, tag="bi")
            nc.gpsimd.dma_start(bi[:], bib[:])

            groups = [(mi, nb0, nbw) for mi in range(n_mi)
                      for nb0, nbw in nbs]
            n_mm = 2 * kb2 + kf  # MM issue slots per group

            def evict(ps, mi, nb0, nbw, ms):
                osb = opool.tile([P, nbw], dt.float32, tag="o")
                nc.vector.tensor_tensor(osb[:], ps[:], sc[:, nb0:nb0 + nbw],
                                        op=Alu.mult)
                nc.vector.tensor_tensor(osb[:], osb[:], bi[:, nb0:nb0 + nbw],
                                        op=Alu.add)
                r0 = ms * m_sup + mi * P
                nc.sync.dma_start(y[r0:r0 + P, nb0:nb0 + nbw], osb[:])

            def issue_mm(ps, ct, mi, nb0, nbw, xbi, xfi, start, stop):
                """MMs for byte-tile ct into psum ps. Returns #MMs issued."""
                if ct < kb2:
                    for jj in range(2):
                        t = 2 * ct + jj
                        nc.tensor.matmul(
                            ps[:],
                            xbi[t][:, mi * P:mi * P + P],
                            wb[t][:, nb0:nb0 + nbw],
                            start=start and jj == 0,
                            stop=stop and jj == 1,
                        )
                else:
                    i = ct - kb2
                    nc.tensor.matmul(
                        ps[:],
                        xfi[i][:, :, mi * P:mi * P + P],
                        w8[i][:, :, nb0:nb0 + nbw],
                        start=start,
                        stop=stop,
                        perf_mode=mybir.MatmulPerfMode.DoubleRow,
                    )

            for ms in range(n_ms):
                # x tile DMAs in consumption order (bf16 tiles then fp8)
                xbi = [None] * (2 * kb2)
                xfi = [None] * kf
                for ct in range(NKT):
                    if ct < kb2:
                        for jj in range(2):
                            t = 2 * ct + jj
                            xt_ = xpool.tile([P, m_sup], dt.bfloat16,
                                             tag=f"xb{t}", name=f"xb{ms}_{t}")
                            nc.sync.dma_start(xt_[:], xb[ms, ct, jj])
                            xbi[t] = xt_
                    else:
                        i = ct - kb2
                        xt_ = xpool.tile([P, 2, m_sup], dt.float8e4,
                                         tag=f"xf{i}", name=f"xf{ms}_{i}")
                        nc.sync.dma_start(xt_[:], xf[ms, i])
                        xfi[i] = xt_

                if ms == 0 and len(groups) <= 8:
                    # k-major across psum groups so PE consumes each W tile
                    # as dequant produces it instead of stalling on the set.
                    pss = [pspool.tile([P, nbw], dt.float32, tag="ps",
                                       name=f"ps{g}")
                           for g, (mi, nb0, nbw) in enumerate(groups)]
                    ng = len(groups)
                    for s in range(NKT):
                        for gi in range(ng):
                            g = (gi + s) % ng
                            mi, nb0, nbw = groups[g]
                            issue_mm(pss[g], s, mi, nb0, nbw, xbi, xfi,
                                     start=(s == 0), stop=(s == NKT - 1))
                    for g, (mi, nb0, nbw) in enumerate(groups):
                        evict(pss[g], mi, nb0, nbw, ms)
                else:
                    for mi, nb0, nbw in groups:
                        ps = pspool.tile([P, nbw], dt.float32, tag="ps")
                        for ct in range(NKT):
                            issue_mm(ps, ct, mi, nb0, nbw, xbi, xfi,
                                     start=(ct == 0), stop=(ct == NKT - 1))
                        evict(ps, mi, nb0, nbw, ms)

    nc.compile()
    return nc


def prep_x(x2, kf=KF, m_sup=M_SUP):
    """[M, K] f32 -> (xf fp8 [n_ms,kf,P,2,m_sup], xb bf16 [n_ms,kb2,2,P,m_sup])."""
    M, K = x2.shape
    kb2 = NKT - kf
    n_ms = M // m_sup
    # k = 256*ct + 2*p + j
    xv = x2.reshape(n_ms, m_sup, NKT, P, 2)
    xf = np.ascontiguousarray(
        xv[:, :, kb2:, :, :].transpose(0, 2, 3, 4, 1)).astype(FP8)
    xb = np.ascontiguousarray(
        xv[:, :, :kb2, :, :].transpose(0, 2, 4, 3, 1)).astype(BF16)
    return xf, xb


def prep_q(q_u8_shard):
    """[NSH, KP] uint8 -> [NKT, P, NSH] int8 (transposed packed bytes)."""
    NSH, KP = q_u8_shard.shape
    return np.ascontiguousarray(q_u8_shard.T).view(np.int8).reshape(
        KP // P, P, NSH)


def prep_bcast(v):
    """[NSH] f32 -> [P, NSH] f32 broadcast tile."""
    return np.ascontiguousarray(
        np.broadcast_to(v.astype(np.float32)[None, :], (P, v.shape[0])))


def _ensure_ntff_hook():
    """Register the axon NTFF profiling hook if the image's antenv lacks
    axon_hooks (trn_boot degrades silently in that case)."""
    try:
        from antenv.axon_hooks import get_axon_ntff_profile_hook  # noqa: F401
        return
    except ImportError:
        pass
    import types
    import antenv
    mod = types.ModuleType("antenv.axon_hooks")
    _h = {"hook": None}
    mod.set_axon_ntff_profile_hook = lambda h: _h.__setitem__("hook", h)
    mod.get_axon_ntff_profile_hook = lambda: _h["hook"]
    sys.modules["antenv.axon_hooks"] = mod
    antenv.axon_hooks = mod
    try:
        from trn_agent_boot.trn_boot import _ntff_profile_via_ctypes
        hook = _ntff_profile_via_ctypes("/opt/axon/libaxon_pjrt.so")
        if hook is not None:
            mod.set_axon_ntff_profile_hook(hook)
    except Exception as e:  # profiling optional; run still works
        print("ntff hook setup failed:", e)


_NC_CACHE = {}


def _get_nc():
    key = (M_FULL, N_SH, KF, M_SUP)
    if key not in _NC_CACHE:
        _NC_CACHE[key] = build_nc(M_FULL, N_SH, kf=KF, m_sup=M_SUP)
    return _NC_CACHE[key]


LAST_RESULT = None


def kernel(x, qweight_packed, w_scales, bias, _profile=False):
    global LAST_RESULT
    x = np.asarray(x)
    qweight_packed = np.asarray(qweight_packed)
    w_scales = np.asarray(w_scales)
    bias = np.asarray(bias)

    # Always shim the profiling hook module: run_bass_kernel_spmd imports
    # it whenever tracing is requested (including via env BASS_TRACE).
    _ensure_ntff_hook()

    nc = _get_nc()

    x2 = np.ascontiguousarray(x.reshape(M_FULL, K_FULL).astype(np.float32))
    xf, xb = prep_x(x2)
    q_u8 = qweight_packed.astype(np.uint8)
    scales_flat = w_scales.reshape(N_FULL)
    bias_flat = bias.reshape(N_FULL)

    in_maps = []
    for c in range(N_CORES):
        r0, r1 = c * N_SH, (c + 1) * N_SH
        in_maps.append({
            "xf": xf,
            "xb": xb,
            "qt": prep_q(q_u8[r0:r1]),
            "scb": prep_bcast(scales_flat[r0:r1]),
            "bib": prep_bcast(bias_flat[r0:r1]),
        })

    res = run_bass_kernel_spmd(nc, in_maps, list(range(N_CORES)),
                               trace=_profile)
    LAST_RESULT = res
    y = np.concatenate([res.results[c]["y"] for c in range(N_CORES)], axis=1)
    return y.reshape(B, S, N_FULL)
